# revision 24
# baseline (speedup 1.0000x reference)
"""DistributionMaxPool Trainium2 kernel (v6).

Math insight: the reference's CxC conv sums the selected 2x2-strided pixel
over ALL input channels and replicates across output channels. Every
per-channel value after that reduction is identical, so the whole
Gaussian-max pipeline runs on channel-summed planes, and the 128-way
channel replication of the output is a pure host-side broadcast view --
the device never materializes it.

Per core per rep the kernel moves 16 MiB in + 32 KiB out and is
HBM-bandwidth-bound; per the timeline-sim trace every engine SEQUENCER is
near-saturated, so the structure minimizes per-sequencer instruction work
as much as raw engine time:

  1. ONE 4 MiB DMA per batch loads both planes [128, 2*4096], alternating
     between the two HWDGE rings (sync/scalar). SWDGE loads sim slower
     (completion latency on the PE-critical path) -- keep them HWDGE.
  2. Channel sums via pixels-stationary matmuls: lhsT = single-stride
     (step 2) view of the plane covering one s-parity of 4 consecutive
     rows (128 pixels), rhs = ones [128, 1], N=1. 64 matmuls per batch
     fill one [128, 32] PSUM tile per plane, evacuated to SBUF by ONE
     3-D-AP DVE copy per (batch, plane): partition p = 64u + 32r + j,
     col 16s + q -> pixel (i = 2q + u, j).
  3. Gaussian math for ALL 4 batches at once: ONE wide stage-A call
     ([128, 64], s=0 vs s=1) and ONE wide stage-B call ([128, 32], r=0 vs
     r=1, batches stacked on partitions p = 32b + j, col = i = 2q + u).
     DVE does arithmetic (rsqrt via bit-trick + 1 Newton step, ~5e-6 rel),
     ACT does erf / tanh / square -- all in one activation table.
  4. Emission order per rep: [4 loads (pure DMA)] -> [PREVIOUS rep's
     math + stores] -> [this rep's PE sums + DVE evacs]. The math's
     inputs are ready at rep start, so DVE/ACT run it immediately and the
     two [128, 32] HWDGE stores (one per ring, y layout [pl, 32b + j, i])
     have their sem-waits resolved before the rings reach them; the next
     rep's loads are never queued behind math-dependent instructions.
     The host untangles b/j/i and broadcasts across channels for free.
"""

import sys

if "/opt/trn_rl_repo" not in sys.path:
    sys.path.insert(0, "/opt/trn_rl_repo")

import numpy as np

B_FULL = 32
N_CORES = 8
B = B_FULL // N_CORES  # 4 batches per core
C = 128
H = W = 64
HO = WO = 32
NPIX = HO * WO  # 1024

EPS = 1e-8
INV_SQRT2 = float(1.0 / np.sqrt(2.0))
INV_SQRT_2PI = float(1.0 / np.sqrt(2.0 * np.pi))
RSQRT_MAGIC = 0x5F3759DF

_CACHE = {}


def _gauss_max(nc, pool, m1, v1, m2, v2, out_mean, out_var, P, F):
    """mean/var of max of two Gaussians, elementwise on [P, F] views.

    d = m1-m2, p = m1+m2, s = v1+v2+eps, rs = 1/sqrt(s), alpha = s*rs,
    beta = d*rs, e = erf(beta/sqrt2), g = exp(-beta^2/2):
      mean = p/2 + e*d/2 + c2*alpha*g
      var  = s/2 + (p^2+d^2)/4 + eps + e*(d*p + (v1-v2))/2 + c2*p*alpha*g
             - mean^2
    ACT ops: erf, tanh, square only (single activation table).
    """
    import concourse.mybir as mybir

    f32 = mybir.dt.float32
    i32 = mybir.dt.int32
    Act = mybir.ActivationFunctionType
    mult = mybir.AluOpType.mult
    add = mybir.AluOpType.add
    shr = mybir.AluOpType.arith_shift_right

    def t(name, dtype=f32):
        return pool.tile([P, F], dtype, name=name, tag=f"{name}_{P}x{F}")

    s_ = t("gs")
    nc.vector.scalar_tensor_tensor(s_[:], v1, EPS, v2, add, add)
    # rs = rsqrt(s): bit-trick seed (1.75e-3 rel) + 1 Newton iteration
    # (-> 4.6e-6 rel; amplified ~65x through the var cancellation that is
    # still ~3e-4, far under the 2e-2 gate)
    sh = t("gsh", i32)
    nc.vector.tensor_scalar(sh[:], s_[:].bitcast(i32), 1, None, shr)
    yi = t("gy", i32)
    nc.vector.tensor_scalar(yi[:], sh[:], -1, RSQRT_MAGIC, mult, add)
    yf = yi[:].bitcast(f32)
    nt1 = t("gnt1")
    nt2 = t("gnt2")
    for _ in range(1):
        nc.vector.tensor_mul(nt1[:], yf, yf)
        nc.vector.scalar_tensor_tensor(nt2[:], nt1[:], -0.5, s_[:], mult, mult)
        nc.vector.scalar_tensor_tensor(yf, nt2[:], 1.5, yf, add, mult)
    alpha = t("galpha")
    nc.vector.tensor_mul(alpha[:], s_[:], yf)
    d = t("gd")
    nc.vector.tensor_sub(d[:], m1, m2)
    beta = t("gbeta")
    nc.vector.tensor_mul(beta[:], d[:], yf)
    e = t("ge")
    nc.scalar.activation(e[:], beta[:], Act.Erf, scale=INV_SQRT2)
    b2 = t("gb2")
    nc.scalar.square(b2[:], beta[:])
    # g = exp(-b2/2) = (1-T)/(1+T), T = tanh(b2/4)
    T = t("gT")
    nc.scalar.activation(T[:], b2[:], Act.Tanh, scale=0.25)
    num = t("gnum")
    nc.vector.tensor_scalar(num[:], T[:], -1.0, 1.0, mult, add)
    den = t("gden")
    nc.vector.tensor_scalar(den[:], T[:], 1.0, None, add)
    dr = t("gdr")
    nc.vector.reciprocal(dr[:], den[:])
    g = t("gg")
    nc.vector.tensor_mul(g[:], num[:], dr[:])

    p_ = t("gp")
    nc.vector.tensor_add(p_[:], m1, m2)
    ag = t("gag")
    nc.vector.tensor_mul(ag[:], alpha[:], g[:])
    # mean = 0.5*p + 0.5*e*d + c2*ag
    u_ = t("gu")
    nc.vector.scalar_tensor_tensor(u_[:], e[:], 0.5, d[:], mult, mult)
    w_ = t("gw")
    nc.vector.scalar_tensor_tensor(w_[:], p_[:], 0.5, u_[:], mult, add)
    nc.vector.scalar_tensor_tensor(out_mean, ag[:], INV_SQRT_2PI, w_[:], mult, add)
    # var
    dv = t("gdv")
    nc.vector.tensor_sub(dv[:], v1, v2)
    dp = t("gdp")
    nc.vector.tensor_mul(dp[:], d[:], p_[:])
    z = t("gz")
    nc.vector.tensor_add(z[:], dp[:], dv[:])
    ez = t("gez")
    nc.vector.scalar_tensor_tensor(ez[:], e[:], 0.5, z[:], mult, mult)
    d2 = t("gd2")
    nc.scalar.square(d2[:], d[:])
    p2 = t("gp2")
    nc.scalar.square(p2[:], p_[:])
    pd = t("gpd")
    nc.vector.tensor_add(pd[:], p2[:], d2[:])
    qd = t("gqd")
    nc.vector.tensor_scalar(qd[:], pd[:], 0.25, EPS, mult, add)
    acc = t("gacc")
    nc.vector.scalar_tensor_tensor(acc[:], s_[:], 0.5, qd[:], mult, add)
    v3 = t("gv3")
    nc.vector.tensor_add(v3[:], ez[:], acc[:])
    pag = t("gpag")
    nc.vector.tensor_mul(pag[:], p_[:], ag[:])
    v4 = t("gv4")
    nc.vector.scalar_tensor_tensor(v4[:], pag[:], INV_SQRT_2PI, v3[:], mult, add)
    v5 = t("gv5")
    nc.scalar.square(v5[:], out_mean)
    nc.vector.scalar_tensor_tensor(out_var, v5[:], -1.0, v4[:], mult, add)


def _loads(nc, x, xin):
    """Emit one rep's 4 batch loads (DMA only -- nothing math-dependent
    sits between them on the sync/scalar streams). Returns the xt tiles."""
    import concourse.mybir as mybir

    f32 = mybir.dt.float32
    xts = []
    for b in range(B):
        # One 4 MiB load: both planes of batch b as [128, 8192]
        # (col = pl*4096 + pix). b0/b1 ride the two HWDGE rings; b2/b3 go
        # SWDGE (gpsimd) -- descriptor generation there is ~1 us per load
        # vs ~4.7 us of HWDGE gen on the sync/scalar sequencers, and all
        # three paths share the same 16 SDMA engines.
        xt = xin.tile([C, 8192], f32, name="xt", tag="xt")
        eng = (nc.sync, nc.scalar, nc.sync, nc.scalar)[b]
        eng.dma_start(
            xt[:].rearrange("c (pl pix) -> c pl pix", pl=2),
            x[b].rearrange("pl c h w -> c pl (h w)"),
        )
        xts.append(xt)
    return xts


def _sums(nc, xts, ones, sums, psp):
    """Per-batch channel sums via pixels-stationary matmuls.

    Returns the (sm_all, sv_all) wide sum tiles:
      partition p = 64u + 32r + j, col = 64s + 16b + q
      -> channel sum at input pixel (h = 2(2q+u) + r, w = 2j + s).
    """
    import concourse.mybir as mybir

    f32 = mybir.dt.float32
    # sm_all/sv_all cols = 64s + 16b + q so stage-A operands are contiguous
    # [128, 64] halves (s=0 | s=1).
    sm_all = sums.tile([128, 128], f32, name="sm", tag="sm")
    sv_all = sums.tile([128, 128], f32, name="sv", tag="sv")
    for b in range(B):
        x5 = xts[b][:].rearrange(
            "c (pl q m s) -> c pl q m s", pl=2, q=16, m=128, s=2
        )
        # 32 matmuls per plane -> [128, 32] PSUM (cols 16s+q), evacuated
        # per s-half into the wide layout by DVE copies.
        for pl in (1, 0):
            ps = psp.tile([128, 32], f32, name="ps", tag="ps")
            for s in range(2):
                for q in range(16):
                    nc.tensor.matmul(
                        ps[:, 16 * s + q : 16 * s + q + 1],
                        x5[:, pl, q, :, s],
                        ones[:, 0:1],
                        start=True,
                        stop=True,
                    )
            dst = sm_all if pl == 0 else sv_all
            # ONE copy per (b, pl): dst cols {64s + 16b + q} as a 3-D view.
            nc.vector.tensor_copy(
                dst[:].rearrange("p (s blk q) -> p s blk q", s=2, blk=4)[
                    :, :, b, :
                ],
                ps[:].rearrange("p (s q) -> p s q", s=2),
            )
    return sm_all, sv_all


def _math_and_store(nc, y, math_pool, state):
    """Gaussian math for one rep's 4 batches (two wide calls) + the two
    16 KiB result stores on the SWDGE (gpsimd) ring. Emitted one rep
    BEHIND the load/sum stream so the in-order sync/scalar engines never
    queue a math-dependent instruction ahead of the next rep's loads.
    """
    import concourse.mybir as mybir

    f32 = mybir.dt.float32
    sm_all, sv_all = state
    # Stage A (all batches): s=0 vs s=1, one [128, 64] call.
    # partition p = 64u + 32r + j; col (within half) = 16b + q;
    # pixel (i = 2q + u, j).
    hm = math_pool.tile([128, 64], f32, name="hm", tag="hm")
    hv = math_pool.tile([128, 64], f32, name="hv", tag="hv")
    _gauss_max(
        nc, math_pool,
        sm_all[:, 0:64], sv_all[:, 0:64], sm_all[:, 64:128], sv_all[:, 64:128],
        hm[:], hv[:], 128, 64,
    )
    # Stage B (all batches, both u halves, ONE call): r=0 vs r=1.
    # Operand layout: partition p = 32b + j, col = i = 2q + u. The align
    # copies write stride-2 single-dim destinations (HW-verified pattern)
    # and read [32, 16] blocks of hm/hv (2-src ops need equal source base
    # partitions, which the m?c tiles provide).
    m1c = math_pool.tile([128, 32], f32, name="m1c", tag="m1c")
    v1c = math_pool.tile([128, 32], f32, name="v1c", tag="v1c")
    m2c = math_pool.tile([128, 32], f32, name="m2c", tag="m2c")
    v2c = math_pool.tile([128, 32], f32, name="v2c", tag="v2c")
    for b in range(B):
        for u in range(2):
            base = 64 * u
            cols = slice(16 * b, 16 * b + 16)

            def dst3(t):
                return t[32 * b : 32 * b + 32, :].rearrange(
                    "p (q u) -> p q u", u=2
                )[:, :, u]

            nc.vector.tensor_copy(dst3(m1c), hm[base : base + 32, cols])
            nc.vector.tensor_copy(dst3(v1c), hv[base : base + 32, cols])
            nc.vector.tensor_copy(dst3(m2c), hm[base + 32 : base + 64, cols])
            nc.vector.tensor_copy(dst3(v2c), hv[base + 32 : base + 64, cols])
    meant = math_pool.tile([128, 32], f32, name="meant", tag="meant")
    vart = math_pool.tile([128, 32], f32, name="vart", tag="vart")
    _gauss_max(
        nc, math_pool,
        m1c[:], v1c[:], m2c[:], v2c[:],
        meant[:], vart[:], 128, 32,
    )
    # Stores: meant/vart hold [p = 32b + j, col = i]; y is [2, 128, 32]
    # = [pl, 32b + j, i] so each plane is ONE contiguous [128, 32] HWDGE
    # store (the host untangles b/j/i while broadcasting). One store per
    # ring; they enqueue behind the current rep's loads and their math
    # dependency resolves early in the rep, so the rings never bubble.
    nc.sync.dma_start(y[0], meant[:])
    nc.scalar.dma_start(y[1], vart[:])


def _build(reps=1):
    import concourse.bacc as bacc
    import concourse.mybir as mybir
    import concourse.tile as tile

    f32 = mybir.dt.float32
    nc = bacc.Bacc("TRN2", target_bir_lowering=False, debug=False, num_devices=N_CORES)

    x = nc.declare_dram_parameter("x", [B, 2, C, H, W], f32, isOutput=False)
    # y[pl, 32b + j, i] -- each plane is one contiguous [128, 32] store
    # (the host untangles the layout while broadcasting). 32 KiB per core.
    y = nc.declare_dram_parameter("y", [2, 4 * WO, HO], f32, isOutput=True)

    with tile.TileContext(nc) as tc:
        with (
            tc.tile_pool(name="xin", bufs=5) as xin,
            tc.tile_pool(name="const", bufs=1) as const,
            tc.tile_pool(name="sums", bufs=2) as sums,
            tc.tile_pool(name="math", bufs=2) as math_pool,
            tc.tile_pool(name="ps", bufs=4, space="PSUM") as psp,
        ):
            ones = const.tile([128, 1], f32)
            nc.gpsimd.memset(ones[:], 1.0)

            # Per rep: loads first (pure DMA), then the PREVIOUS rep's
            # math+stores (inputs ready at rep start, so DVE/ACT run it
            # immediately and the stores' sem-waits resolve before the
            # rings reach them), then this rep's PE sums + DVE evacs.
            prev = None
            for _rep in range(reps):
                xts = _loads(nc, x, xin)
                if prev is not None:
                    _math_and_store(nc, y, math_pool, prev)
                prev = _sums(nc, xts, ones, sums, psp)
            _math_and_store(nc, y, math_pool, prev)

    nc.compile()
    return nc


def _get_nc():
    if "nc" not in _CACHE:
        _CACHE["nc"] = _build()
    return _CACHE["nc"]


def kernel(x: np.ndarray) -> np.ndarray:
    from concourse.bass_utils import run_bass_kernel_spmd

    assert x.shape == (B_FULL, 2, C, H, W), x.shape
    x = np.ascontiguousarray(x, dtype=np.float32)
    nc = _get_nc()
    in_maps = [{"x": x[i * B : (i + 1) * B]} for i in range(N_CORES)]
    res = run_bass_kernel_spmd(nc, in_maps, list(range(N_CORES)))
    # per-core y: [pl, 32b + j, i] -> [b, pl, i, j]
    yt = np.stack(
        [
            res.results[i]["y"].reshape(2, B, WO, HO).transpose(1, 0, 3, 2)
            for i in range(N_CORES)
        ]
    ).reshape(B_FULL, 2, HO, WO)
    full = np.broadcast_to(
        yt[:, :, None, :, :], (B_FULL, 2, C, HO, WO)
    )
    return np.ascontiguousarray(full)


# revision 29
# speedup vs baseline: 1.5735x; 1.5735x over previous
"""DistributionMaxPool Trainium2 kernel (v6).

Math insight: the reference's CxC conv sums the selected 2x2-strided pixel
over ALL input channels and replicates across output channels. Every
per-channel value after that reduction is identical, so the whole
Gaussian-max pipeline runs on channel-summed planes, and the 128-way
channel replication of the output is a pure host-side broadcast view --
the device never materializes it.

Per core per rep the kernel moves 16 MiB in + 32 KiB out and is
HBM-bandwidth-bound; per the timeline-sim trace every engine SEQUENCER is
near-saturated, so the structure minimizes per-sequencer instruction work
as much as raw engine time:

  1. ONE 4 MiB DMA per batch loads both planes [128, 2*4096], alternating
     between the two HWDGE rings (sync/scalar). SWDGE loads sim slower
     (completion latency on the PE-critical path) -- keep them HWDGE.
  2. Channel sums via pixels-stationary matmuls: lhsT = single-stride
     (step 2) view of the plane covering one s-parity of 4 consecutive
     rows (128 pixels), rhs = ones [128, 1], N=1. 64 matmuls per batch
     fill one [128, 32] PSUM tile per plane, evacuated to SBUF by ONE
     3-D-AP DVE copy per (batch, plane): partition p = 64u + 32r + j,
     col 16s + q -> pixel (i = 2q + u, j).
  3. Gaussian math for ALL 4 batches at once: ONE wide stage-A call
     ([128, 64], s=0 vs s=1) and ONE wide stage-B call ([128, 32], r=0 vs
     r=1, batches stacked on partitions p = 32b + j, col = i = 2q + u).
     DVE does arithmetic (rsqrt via bit-trick + 1 Newton step, ~5e-6 rel),
     ACT does erf / tanh / square -- all in one activation table.
  4. Emission order per rep: [4 loads (pure DMA)] -> [PREVIOUS rep's
     math + stores] -> [this rep's PE sums + DVE evacs]. The math's
     inputs are ready at rep start, so DVE/ACT run it immediately and the
     two [128, 32] HWDGE stores (one per ring, y layout [pl, 32b + j, i])
     have their sem-waits resolved before the rings reach them; the next
     rep's loads are never queued behind math-dependent instructions.
     The host untangles b/j/i and broadcasts across channels for free.
"""

import sys

if "/opt/trn_rl_repo" not in sys.path:
    sys.path.insert(0, "/opt/trn_rl_repo")

import numpy as np

B_FULL = 32
N_CORES = 8
B = B_FULL // N_CORES  # 4 batches per core
C = 128
H = W = 64
HO = WO = 32
NPIX = HO * WO  # 1024

EPS = 1e-8
INV_SQRT2 = float(1.0 / np.sqrt(2.0))
INV_SQRT_2PI = float(1.0 / np.sqrt(2.0 * np.pi))
RSQRT_MAGIC = 0x5F3759DF

_CACHE = {}


def _gauss_max(nc, pool, m1, v1, m2, v2, out_mean, out_var, P, F):
    """mean/var of max of two Gaussians, elementwise on [P, F] views.

    d = m1-m2, p = m1+m2, s = v1+v2+eps, rs = 1/sqrt(s), alpha = s*rs,
    beta = d*rs, e = erf(beta/sqrt2), g = exp(-beta^2/2):
      mean = p/2 + e*d/2 + c2*alpha*g
      var  = s/2 + (p^2+d^2)/4 + eps + e*(d*p + (v1-v2))/2 + c2*p*alpha*g
             - mean^2
    ACT ops: erf, tanh, square only (single activation table).
    """
    import concourse.mybir as mybir

    f32 = mybir.dt.float32
    i32 = mybir.dt.int32
    Act = mybir.ActivationFunctionType
    mult = mybir.AluOpType.mult
    add = mybir.AluOpType.add
    shr = mybir.AluOpType.arith_shift_right

    def t(name, dtype=f32):
        return pool.tile([P, F], dtype, name=name, tag=f"{name}_{P}x{F}")

    s_ = t("gs")
    nc.vector.scalar_tensor_tensor(s_[:], v1, EPS, v2, add, add)
    # rs = rsqrt(s): bit-trick seed (1.75e-3 rel) + 1 Newton iteration
    # (-> 4.6e-6 rel; amplified ~65x through the var cancellation that is
    # still ~3e-4, far under the 2e-2 gate)
    sh = t("gsh", i32)
    nc.vector.tensor_scalar(sh[:], s_[:].bitcast(i32), 1, None, shr)
    yi = t("gy", i32)
    nc.vector.tensor_scalar(yi[:], sh[:], -1, RSQRT_MAGIC, mult, add)
    yf = yi[:].bitcast(f32)
    nt1 = t("gnt1")
    nt2 = t("gnt2")
    for _ in range(1):
        nc.vector.tensor_mul(nt1[:], yf, yf)
        nc.vector.scalar_tensor_tensor(nt2[:], nt1[:], -0.5, s_[:], mult, mult)
        nc.vector.scalar_tensor_tensor(yf, nt2[:], 1.5, yf, add, mult)
    alpha = t("galpha")
    nc.vector.tensor_mul(alpha[:], s_[:], yf)
    d = t("gd")
    nc.vector.tensor_sub(d[:], m1, m2)
    beta = t("gbeta")
    nc.vector.tensor_mul(beta[:], d[:], yf)
    e = t("ge")
    nc.scalar.activation(e[:], beta[:], Act.Erf, scale=INV_SQRT2)
    b2 = t("gb2")
    nc.scalar.square(b2[:], beta[:])
    # g = exp(-b2/2) = (1-T)/(1+T), T = tanh(b2/4)
    T = t("gT")
    nc.scalar.activation(T[:], b2[:], Act.Tanh, scale=0.25)
    num = t("gnum")
    nc.vector.tensor_scalar(num[:], T[:], -1.0, 1.0, mult, add)
    den = t("gden")
    nc.vector.tensor_scalar(den[:], T[:], 1.0, None, add)
    dr = t("gdr")
    nc.vector.reciprocal(dr[:], den[:])
    g = t("gg")
    nc.vector.tensor_mul(g[:], num[:], dr[:])

    p_ = t("gp")
    nc.vector.tensor_add(p_[:], m1, m2)
    ag = t("gag")
    nc.vector.tensor_mul(ag[:], alpha[:], g[:])
    # mean = 0.5*p + 0.5*e*d + c2*ag
    u_ = t("gu")
    nc.vector.scalar_tensor_tensor(u_[:], e[:], 0.5, d[:], mult, mult)
    w_ = t("gw")
    nc.vector.scalar_tensor_tensor(w_[:], p_[:], 0.5, u_[:], mult, add)
    nc.vector.scalar_tensor_tensor(out_mean, ag[:], INV_SQRT_2PI, w_[:], mult, add)
    # var
    dv = t("gdv")
    nc.vector.tensor_sub(dv[:], v1, v2)
    dp = t("gdp")
    nc.vector.tensor_mul(dp[:], d[:], p_[:])
    z = t("gz")
    nc.vector.tensor_add(z[:], dp[:], dv[:])
    ez = t("gez")
    nc.vector.scalar_tensor_tensor(ez[:], e[:], 0.5, z[:], mult, mult)
    d2 = t("gd2")
    nc.scalar.square(d2[:], d[:])
    p2 = t("gp2")
    nc.scalar.square(p2[:], p_[:])
    pd = t("gpd")
    nc.vector.tensor_add(pd[:], p2[:], d2[:])
    qd = t("gqd")
    nc.vector.tensor_scalar(qd[:], pd[:], 0.25, EPS, mult, add)
    acc = t("gacc")
    nc.vector.scalar_tensor_tensor(acc[:], s_[:], 0.5, qd[:], mult, add)
    v3 = t("gv3")
    nc.vector.tensor_add(v3[:], ez[:], acc[:])
    pag = t("gpag")
    nc.vector.tensor_mul(pag[:], p_[:], ag[:])
    v4 = t("gv4")
    nc.vector.scalar_tensor_tensor(v4[:], pag[:], INV_SQRT_2PI, v3[:], mult, add)
    v5 = t("gv5")
    nc.scalar.square(v5[:], out_mean)
    nc.vector.scalar_tensor_tensor(out_var, v5[:], -1.0, v4[:], mult, add)


def _loads(nc, x, xin):
    """Emit one rep's 4 batch loads (DMA only -- nothing math-dependent
    sits between them on the sync/scalar streams). Returns the xt tiles."""
    import concourse.mybir as mybir

    f32 = mybir.dt.float32
    xts = []
    for b in range(B):
        # One 4 MiB load: both planes of batch b as [128, 8192]
        # (col = pl*4096 + pix), alternating the two HWDGE rings. Simmed
        # and rejected: SWDGE loads (completion latency lands on the
        # PE-critical path) and per-plane 2 MiB splits (more ring work
        # for no gain).
        xt = xin.tile([C, 8192], f32, name="xt", tag="xt")
        eng = (nc.sync, nc.scalar, nc.sync, nc.scalar)[b]
        eng.dma_start(
            xt[:].rearrange("c (pl pix) -> c pl pix", pl=2),
            x[b].rearrange("pl c h w -> c pl (h w)"),
        )
        xts.append(xt)
    return xts


def _sums(nc, xts, ones, sums, psp):
    """Per-batch channel sums via pixels-stationary matmuls.

    Returns the (sm_all, sv_all) wide sum tiles:
      partition p = 64u + 32r + j, col = 64s + 16b + q
      -> channel sum at input pixel (h = 2(2q+u) + r, w = 2j + s).
    """
    import concourse.mybir as mybir

    f32 = mybir.dt.float32
    # sm_all/sv_all cols = 64s + 16b + q so stage-A operands are contiguous
    # [128, 64] halves (s=0 | s=1).
    sm_all = sums.tile([128, 128], f32, name="sm", tag="sm")
    sv_all = sums.tile([128, 128], f32, name="sv", tag="sv")
    for b in range(B):
        x5 = xts[b][:].rearrange(
            "c (pl q m s) -> c pl q m s", pl=2, q=16, m=128, s=2
        )
        # 32 matmuls per plane -> [128, 32] PSUM (cols 16s+q), evacuated
        # per s-half into the wide layout by DVE copies.
        for pl in (1, 0):
            ps = psp.tile([128, 32], f32, name="ps", tag="ps")
            for s in range(2):
                for q in range(16):
                    nc.tensor.matmul(
                        ps[:, 16 * s + q : 16 * s + q + 1],
                        x5[:, pl, q, :, s],
                        ones[:, 0:1],
                        start=True,
                        stop=True,
                    )
            dst = sm_all if pl == 0 else sv_all
            # ONE copy per (b, pl): dst cols {64s + 16b + q} as a 3-D view.
            nc.vector.tensor_copy(
                dst[:].rearrange("p (s blk q) -> p s blk q", s=2, blk=4)[
                    :, :, b, :
                ],
                ps[:].rearrange("p (s q) -> p s q", s=2),
            )
    return sm_all, sv_all


def _math_and_store(nc, y, math_pool, state):
    """Gaussian math for one rep's 4 batches (two wide calls) + the two
    16 KiB result stores on the SWDGE (gpsimd) ring. Emitted one rep
    BEHIND the load/sum stream so the in-order sync/scalar engines never
    queue a math-dependent instruction ahead of the next rep's loads.
    """
    import concourse.mybir as mybir

    f32 = mybir.dt.float32
    sm_all, sv_all = state
    # Stage A (all batches): s=0 vs s=1, one [128, 64] call.
    # partition p = 64u + 32r + j; col (within half) = 16b + q;
    # pixel (i = 2q + u, j).
    hm = math_pool.tile([128, 64], f32, name="hm", tag="hm")
    hv = math_pool.tile([128, 64], f32, name="hv", tag="hv")
    _gauss_max(
        nc, math_pool,
        sm_all[:, 0:64], sv_all[:, 0:64], sm_all[:, 64:128], sv_all[:, 64:128],
        hm[:], hv[:], 128, 64,
    )
    # Stage B (all batches, both u halves, ONE call): r=0 vs r=1.
    # Operand layout: partition p = 32b + j, col = i = 2q + u. The align
    # copies write stride-2 single-dim destinations (HW-verified pattern)
    # and read [32, 16] blocks of hm/hv (2-src ops need equal source base
    # partitions, which the m?c tiles provide).
    m1c = math_pool.tile([128, 32], f32, name="m1c", tag="m1c")
    v1c = math_pool.tile([128, 32], f32, name="v1c", tag="v1c")
    m2c = math_pool.tile([128, 32], f32, name="m2c", tag="m2c")
    v2c = math_pool.tile([128, 32], f32, name="v2c", tag="v2c")
    for b in range(B):
        for u in range(2):
            base = 64 * u
            cols = slice(16 * b, 16 * b + 16)

            def dst3(t):
                return t[32 * b : 32 * b + 32, :].rearrange(
                    "p (q u) -> p q u", u=2
                )[:, :, u]

            nc.vector.tensor_copy(dst3(m1c), hm[base : base + 32, cols])
            nc.vector.tensor_copy(dst3(v1c), hv[base : base + 32, cols])
            nc.vector.tensor_copy(dst3(m2c), hm[base + 32 : base + 64, cols])
            nc.vector.tensor_copy(dst3(v2c), hv[base + 32 : base + 64, cols])
    meant = math_pool.tile([128, 32], f32, name="meant", tag="meant")
    vart = math_pool.tile([128, 32], f32, name="vart", tag="vart")
    _gauss_max(
        nc, math_pool,
        m1c[:], v1c[:], m2c[:], v2c[:],
        meant[:], vart[:], 128, 32,
    )
    # Stores: meant/vart hold [p = 32b + j, col = i]; y is [2, 128, 32]
    # = [pl, 32b + j, i] so each plane is ONE contiguous [128, 32] HWDGE
    # store (the host untangles b/j/i while broadcasting). One store per
    # ring; they enqueue behind the current rep's loads and their math
    # dependency resolves early in the rep, so the rings never bubble.
    nc.sync.dma_start(y[0], meant[:])
    nc.scalar.dma_start(y[1], vart[:])


def _build(reps=1):
    import concourse.bacc as bacc
    import concourse.mybir as mybir
    import concourse.tile as tile

    f32 = mybir.dt.float32
    nc = bacc.Bacc("TRN2", target_bir_lowering=False, debug=False, num_devices=N_CORES)

    x = nc.declare_dram_parameter("x", [B, 2, C, H, W], f32, isOutput=False)
    # y[pl, 32b + j, i] -- each plane is one contiguous [128, 32] store
    # (the host untangles the layout while broadcasting). 32 KiB per core.
    y = nc.declare_dram_parameter("y", [2, 4 * WO, HO], f32, isOutput=True)

    with tile.TileContext(nc) as tc:
        with (
            tc.tile_pool(name="xin", bufs=5) as xin,
            tc.tile_pool(name="const", bufs=1) as const,
            tc.tile_pool(name="sums", bufs=2) as sums,
            tc.tile_pool(name="math", bufs=2) as math_pool,
            tc.tile_pool(name="ps", bufs=4, space="PSUM") as psp,
        ):
            ones = const.tile([128, 1], f32)
            nc.gpsimd.memset(ones[:], 1.0)

            # Per rep: loads first (pure DMA), then the PREVIOUS rep's
            # math+stores (inputs ready at rep start, so DVE/ACT run it
            # immediately and the stores' sem-waits resolve before the
            # rings reach them), then this rep's PE sums + DVE evacs.
            prev = None
            for _rep in range(reps):
                xts = _loads(nc, x, xin)
                if prev is not None:
                    _math_and_store(nc, y, math_pool, prev)
                prev = _sums(nc, xts, ones, sums, psp)
            _math_and_store(nc, y, math_pool, prev)

    nc.compile()
    return nc


def _get_nc():
    if "nc" not in _CACHE:
        _CACHE["nc"] = _build()
    return _CACHE["nc"]


def kernel(x: np.ndarray) -> np.ndarray:
    from concourse.bass_utils import run_bass_kernel_spmd

    assert x.shape == (B_FULL, 2, C, H, W), x.shape
    x = np.ascontiguousarray(x, dtype=np.float32)
    nc = _get_nc()
    in_maps = [{"x": x[i * B : (i + 1) * B]} for i in range(N_CORES)]
    res = run_bass_kernel_spmd(nc, in_maps, list(range(N_CORES)))
    # per-core y: [pl, 32b + j, i] -> [b, pl, i, j]
    yt = np.stack(
        [
            res.results[i]["y"].reshape(2, B, WO, HO).transpose(1, 0, 3, 2)
            for i in range(N_CORES)
        ]
    ).reshape(B_FULL, 2, HO, WO)
    full = np.broadcast_to(
        yt[:, :, None, :, :], (B_FULL, 2, C, HO, WO)
    )
    return np.ascontiguousarray(full)
